# revision 39
# baseline (speedup 1.0000x reference)
"""Trainium2 Bass kernel for Restormer-style channel attention (XCA).

Reference computation (B=4, C=192, H=W=128, HEADS=4, ch=48):
  qkv = dwconv3x3(conv1x1(x, w_qkv), w_dw)       # [B, 576, H, W]
  q, k, v = split(qkv); per head: L2-normalize q, k over tokens
  attn = softmax(q @ k^T * temp)                 # [B, 4, 48, 48]
  out = conv1x1((attn @ v).reshape, w_proj)

Sharding: 8 cores = 4 batches x 2 row-halves (64 image rows each, +1 halo
row each side).  Per-core gram matrices + channel sumsqs are AllReduce'd
between the two cores of a batch (38 KB), overlapped with the v path.

Depthwise conv runs on the TensorEngine as diagonal matmuls with PSUM
tap-accumulation, reading shifted views of zero-padded pitch-144 buffers.
The q/k path uses fp8 DoubleRow for conv1x1 (channel pairs interleaved
per image row so the DR pair step is 128 and inner runs stay 128 long)
and for the depthwise conv (3 dy-pairs at step RP=144 + 3 singles).
The v path stays bf16 (output noise propagates ~1:1); its conv1x1 runs
once over the 66-row shard into global padded buffers, and every K=64
matmul is zero-padded to K=128 (the PE pays ~100 ns on any matmul
adjacent to a 64-row-config one).
attn@v and the output projection are fused into a single matmul stage
via M = blockdiag(attn) @ WprojT computed on device.  PSUM drains
alternate between ScalarE and VectorE.
"""

import sys

for p in ("/opt/trn_rl_repo", "/root/.axon_site/_ro/trn_rl_repo"):
    if p not in sys.path:
        sys.path.insert(0, p)

import numpy as np
import ml_dtypes
import bass_rust

BF16NP = ml_dtypes.bfloat16

import concourse.bass as bass
import concourse.bacc as bacc
import concourse.mybir as mybir
from concourse.tile import TileContext
from concourse.bass_utils import run_bass_kernel_spmd

F32 = mybir.dt.float32
F32R = mybir.dt.float32r
BF16 = mybir.dt.bfloat16
FP8 = mybir.dt.float8e4
FP8NP = mybir.dt.np(FP8)
AF = mybir.ActivationFunctionType
AX = mybir.AxisListType
ALU = mybir.AluOpType

C = 192
HEADS = 4
CH = 48
W = 128
SH_ROWS = 66          # shard rows incl. 1 halo row each side
OUT_ROWS = 64
BAND = 16             # output rows per band
NBANDS = OUT_ROWS // BAND
RP = 144              # padded row pitch (16-aligned for fp8 DoubleRow)
NTOK = SH_ROWS * W    # 8448 tokens incl halo rows
OTOK = OUT_ROWS * W   # 8192 output tokens
TAPS = [(t // 3, t % 3) for t in range(9)]
AR_N = 9600           # 48*192 gram + 192 qss + 192 kss
# q/k band buffer: 18 padded rows + a replica of rows 2..17 placed at
# offset 18*RP+15 so the (dy2,dx0)/(dy2,dx1) DoubleRow pair step
# (18*RP+15+1-2*RP = 2320) is 16-aligned.
PB_REP = 18 * RP + 15         # replica offset inside pb
PB_W = PB_REP + 16 * RP       # pb tile width (4911)
PB_STEP = PB_REP + 1 - 2 * RP  # 2320, the dy2 dx-pair step


def r32(ap):
    return ap.bitcast(F32R)


def _rawap(ap, pairs, offset):
    c = ap.copy()
    c.ap = bass_rust.VecI64Pair(pairs)
    c.offset = offset
    return c


def _copy(eng, dst, src):
    if getattr(eng, "engine", None) == mybir.EngineType.Activation:
        return eng.copy(dst, src)
    return eng.tensor_copy(dst, src)


def _program(nc, tc, io):
    x_d = io["x"].ap().rearrange("c h w -> c (h w)")          # [192, 8448]
    wqkvT_d = io["wqkvT"].ap()                                # [256, 640]
    dqk8p_d = io["dqk8p"].ap()                                # [128, 9*2*128]
    dqk8d_d = io["dqk8d"].ap()                                # [128, 3*2*128]
    dqk8s_d = io["dqk8s"].ap()                                # [128, 3*128]
    x8_d = io["x8"].ap()                                      # [96, 2*8448]
    wq8_d = io["wq8"].ap()                                    # [96, 2*384]
    dv0_d = io["dv0"].ap()                                    # [128, 9*128]
    dv1pk_d = io["dv1pk"].ap()                                # [128, 3*128]
    dv1s_d = io["dv1s"].ap()                                  # [128, 3*128]
    wprojT_d = io["wprojT"].ap()                              # [192, 192]
    idf_d = io["idf"].ap()                                    # [128, 128]
    tempb_d = io["tempb"].ap()                                # [48, 4]
    out_d = io["out"].ap().rearrange("c h w -> c (h w)")      # [192, 8192]

    CTOK = (BAND + 2) * W            # conv tokens per band (2304)
    NSUB = BAND * W // 512           # dw 512-token subtiles per band (4)
    NCHK = BAND                      # 128-token chunks per band (16)

    with (
        tc.tile_pool(name="sb", bufs=1) as sb,
        tc.tile_pool(name="ps", bufs=1, space="PSUM") as ps,
        tc.tile_pool(name="dram", bufs=1, space="DRAM") as dp,
    ):
        # ---------------- persistent SBUF loads ----------------
        # fp8 x for the q/k conv: pairs (c, c+96) interleaved per image
        # row, so the DoubleRow pair step is 128 (16-aligned) and the
        # moving AP keeps 128-element inner runs (the fast PE rhs shape).
        x8 = sb.tile([96, 2 * NTOK], FP8, tag="x8")
        NXC = 16
        csz = NTOK // NXC
        qs = (nc.sync, nc.scalar)
        wq8 = sb.tile([96, 2 * 384], FP8, tag="wq8")
        nc.scalar.dma_start(wq8, wq8_d)
        # first 9 small chunks cover band 0 quickly, then the q/k dw
        # weights (needed by dwA(0) shortly after), then the rest
        bounds = [0, 256, 512, 768, 1024, 1280, 1536, 1792, 2048, 2304]
        step = (NTOK - 2304) // 8
        bounds += [2304 + step * i for i in range(1, 8)] + [NTOK]
        dqk8p = sb.tile([128, 9 * 256], FP8, tag="dqk8p")
        dqk8d = sb.tile([128, 3 * 256], FP8, tag="dqk8d")
        dqk8s = sb.tile([128, 3 * 128], FP8, tag="dqk8s")
        wqkvT0 = sb.tile([128, 640], BF16, tag="wqkvT0")
        wqkvT1 = sb.tile([128, 640], BF16, tag="wqkvT1")
        for j in range(len(bounds) - 1):
            a, bnd = bounds[j], bounds[j + 1]
            qs[j % 2].dma_start(x8[:, 2 * a:2 * bnd], x8_d[:, 2 * a:2 * bnd])
            if j == 8:
                nc.scalar.dma_start(dqk8p, dqk8p_d)
                nc.scalar.dma_start(dqk8d, dqk8d_d)
                nc.scalar.dma_start(dqk8s, dqk8s_d)
                nc.sync.dma_start(wqkvT0, wqkvT_d[0:128, :])
                nc.sync.dma_start(wqkvT1, wqkvT_d[128:256, :])
        # warm-up runs on a memset dummy so the PE starts before any DMA
        wdum = sb.tile([128, 128], BF16, tag="wdum")
        nc.vector.memset(wdum, 0.0)
        for wi in range(16):
            pw = ps.tile([128, 128], F32, tag="dw", bufs=2, name="pw")
            for wj in range(4):
                nc.tensor.matmul(pw, wdum, wdum, start=(wj == 0),
                                 stop=(wj == 3))
        idf = sb.tile([128, 128], F32, tag="idf")
        nc.gpsimd.dma_start(idf, idf_d)
        idb = sb.tile([128, 128], FP8, tag="idb")
        nc.scalar.copy(idb, idf)
        idb_box = [idb]
        tb = sb.tile([48, 4], F32, tag="tb")
        nc.sync.dma_start(tb, tempb_d)
        dv0 = sb.tile([128, 9 * 128], BF16, tag="dv0")
        nc.gpsimd.dma_start(dv0, dv0_d)
        dv1pk = sb.tile([128, 3 * 128], BF16, tag="dv1pk")
        nc.gpsimd.dma_start(dv1pk, dv1pk_d)
        dv1s = sb.tile([128, 3 * 128], BF16, tag="dv1s")
        nc.gpsimd.dma_start(dv1s, dv1s_d)
        wpT0 = sb.tile([128, 192], BF16, tag="wpT0")
        wpT1 = sb.tile([64, 192], BF16, tag="wpT1")
        nc.gpsimd.dma_start(wpT0, wprojT_d[0:128, :])
        nc.gpsimd.dma_start(wpT1, wprojT_d[128:192, :])
        # bf16 x for the v path; x1's upper 64 partitions are filled with
        # a copy of its lower half (their wqkvT rows are zero) so both
        # conv K-blocks are full 128-row matmuls (the PE pays a ~100 ns
        # penalty on every matmul adjacent to a 64-row one) without
        # DMAing 1 MB of zeros from HBM.  The chunk loads are emitted
        # inside convA(b) (4 per band) so the early DMA engines belong
        # to the latency-critical x8 load.
        x0 = sb.tile([128, NTOK], BF16, tag="x0")
        x1 = sb.tile([128, NTOK], BF16, tag="x1")

        def loadx(j):
            nc.gpsimd.dma_start(x0[:, j * csz:(j + 1) * csz],
                                x_d[0:128, j * csz:(j + 1) * csz])
            nc.gpsimd.dma_start(x1[0:64, j * csz:(j + 1) * csz],
                                x_d[128:192, j * csz:(j + 1) * csz])
            nc.sync.dma_start(x1[64:128, j * csz:(j + 1) * csz],
                              x1[0:64, j * csz:(j + 1) * csz])

        # running per-channel sum-of-squares accumulators, updated on the
        # otherwise-idle gpsimd engine (scalar/vector stay free to drain)
        ssa = [sb.tile([128, 512], F32, tag=f"ssa{j}", name=f"ssa{j}")
               for j in range(3)]
        ssq = [sb.tile([128, 1], F32, tag=f"ssq{j}", name=f"ssq{j}")
               for j in range(3)]

        gram_ps = ps.tile([48, 192], F32, tag="gram")

        # v-path global buffers (allocated early; first conv tiles are
        # emitted inside the phase-A pipeline to fill the gram tail gap)
        vdw0 = sb.tile([128, OTOK], BF16, tag="vdw0")
        vdw1 = sb.tile([128, OTOK], BF16, tag="vdw1")
        cvp0 = sb.tile([128, SH_ROWS * RP], BF16, tag="cvp0")
        cv1d = sb.tile([128, SH_ROWS * RP], BF16, tag="cv1d")
        cvp0_3 = cvp0.rearrange("p (r w) -> p r w", w=RP)
        cv1d_3 = cv1d.rearrange("p (r w) -> p r w", w=RP)
        nc.vector.memset(cvp0_3[:, :, 7:8], 0.0)
        nc.vector.memset(cvp0_3[:, :, 8 + W:9 + W], 0.0)
        nc.vector.memset(cv1d_3[0:64, :, 7:8], 0.0)
        nc.vector.memset(cv1d_3[0:64, :, 8 + W:9 + W], 0.0)

        NVT = (NTOK + 511) // 512     # 17 conv tiles over 66 rows

        def convV(t0, t1):
            for t in range(t0, t1):
                n0 = t * 512
                nsz = min(512, NTOK - n0)
                for j, m0 in enumerate((384, 512)):
                    pc = ps.tile([128, 512], F32, tag="conv", bufs=3)
                    r0v = _rawap(x0, [[NTOK, 128], [128, nsz // 128],
                                      [1, 128]], n0)
                    r1v = _rawap(x1, [[NTOK, 128], [128, nsz // 128],
                                      [1, 128]], n0)
                    nc.tensor.matmul(pc[:, 0:nsz], wqkvT0[:, m0:m0 + 128],
                                     r0v, start=True, stop=False)
                    nc.tensor.matmul(pc[:, 0:nsz], wqkvT1[:, m0:m0 + 128],
                                     r1v, start=False, stop=True)
                    r0 = n0 // W
                    dst3 = cvp0_3 if j == 0 else cv1d_3
                    msz = 128 if j == 0 else 64
                    eng = nc.scalar if (j + t) % 2 == 0 else nc.vector
                    _copy(eng,
                          dst3[0:msz, r0:r0 + nsz // W, 8:8 + W],
                          pc[0:msz, 0:nsz].rearrange("p (r w) -> p r w",
                                                     w=W))

        CONV_NT = tuple((i * 512, min(512, CTOK - i * 512))
                        for i in range((CTOK + 511) // 512))

        # ---------------- Phase A: q/k path, software-pipelined ------
        pb_of, qkdw_of = {}, {}

        wq83 = wq8.rearrange("p (i m) -> p i m", m=384)

        def convA(b):
            for j in range(4 * b, 4 * b + 4):
                loadx(j)
            tok0 = b * BAND * W
            pb = [sb.tile([128, PB_W], FP8, tag=f"pb{j}", bufs=2,
                          name=f"pb{j}") for j in range(3)]
            pb3 = [t[:, 0:18 * RP].rearrange("p (r w) -> p r w", w=RP)
                   for t in pb]
            for j in range(3):
                nc.vector.memset(pb3[j][:, :, 7:8], 0.0)
                nc.vector.memset(pb3[j][:, :, 8 + W:9 + W], 0.0)
            for j in range(3):
                m0 = j * 128
                for n0, nsz in CONV_NT:
                    pc = ps.tile([128, 512], F32, tag="conv", bufs=3)
                    rhs = _rawap(x8, [[2 * NTOK, 96], [128, 2],
                                      [256, nsz // 128], [1, 128]],
                                 2 * (tok0 + n0))
                    nc.tensor.matmul(pc[:, 0:nsz], wq83[:, :, m0:m0 + 128],
                                     rhs, start=True, stop=True,
                                     perf_mode=mybir.MatmulPerfMode.DoubleRow)
                    r0 = n0 // W
                    eng = nc.scalar if (j + n0 // 512) % 2 == 0 else nc.vector
                    _copy(eng,
                          pb3[j][:, r0:r0 + nsz // W, 8:8 + W],
                          pc[:, 0:nsz].rearrange("p (r w) -> p r w", w=W))
            # replicate rows 2..17 at the 16-alignment-fixing offset for
            # the (dy2,dx0)+(dy2,dx1) DoubleRow pair
            for j in range(3):
                (nc.sync, nc.scalar, nc.gpsimd)[j].dma_start(
                    pb[j][:, PB_REP:PB_REP + 16 * RP],
                    pb[j][:, 2 * RP:18 * RP])
            pb_of[b] = pb

        def dwA(b):
            pb = pb_of.pop(b)
            pb3 = [t[:, 0:18 * RP].rearrange("p (r w) -> p r w", w=RP)
                   for t in pb]
            qkdw = [sb.tile([128, BAND * W], FP8, tag=f"qkdw{j}", bufs=2,
                            name=f"qkdw{j}") for j in range(3)]
            for j in range(3):
                for s in range(NSUB):
                    pd = ps.tile([128, 512], F32, tag="dw", bufs=2)
                    for dx in range(3):
                        rhs = _rawap(pb[j],
                                     [[PB_W, 128], [RP, 2], [RP, 4],
                                      [1, W]],
                                     4 * s * RP + 7 + dx)
                        nc.tensor.matmul(
                            pd, dqk8p[:, (j * 3 + dx) * 256:
                                      (j * 3 + dx + 1) * 256]
                            .rearrange("p (i m) -> p i m", m=128),
                            rhs, start=(dx == 0), stop=False,
                            perf_mode=mybir.MatmulPerfMode.DoubleRow)
                    # (dy2,dx0)+(dy2,dx1) in one DoubleRow pass via the
                    # replica region (pair step PB_STEP, 16-aligned)
                    rhs = _rawap(pb[j],
                                 [[PB_W, 128], [PB_STEP, 2], [RP, 4],
                                  [1, W]],
                                 (4 * s + 2) * RP + 7)
                    nc.tensor.matmul(
                        pd, dqk8d[:, j * 256:(j + 1) * 256]
                        .rearrange("p (i m) -> p i m", m=128),
                        rhs, start=False, stop=False,
                        perf_mode=mybir.MatmulPerfMode.DoubleRow)
                    # (dy2,dx2) single
                    rhs = pb3[j][:, 4 * s + 2: 4 * s + 2 + 4,
                                 9: 9 + W]
                    nc.tensor.matmul(
                        pd, dqk8s[:, j * 128:(j + 1) * 128],
                        rhs, start=False, stop=True)
                    eng = nc.scalar if s % 2 == 0 else nc.vector
                    _copy(eng, qkdw[j][:, s * 512:(s + 1) * 512], pd)
                    qs = qkdw[j][:, s * 512:(s + 1) * 512]
                    if b == 0 and s == 0:
                        nc.gpsimd.tensor_mul(ssa[j], qs, qs)
                    else:
                        sq = sb.tile([128, 512], F32, tag="sqs", bufs=2)
                        nc.gpsimd.tensor_mul(sq, qs, qs)
                        nc.gpsimd.tensor_add(ssa[j], ssa[j], sq)
            qkdw_of[b] = qkdw

        def gramA(b):
            qkdw = qkdw_of.pop(b)
            idb = idb_box[0]
            for cp in range(NCHK // 2):
                qt = sb.tile([128, 768], FP8, tag="qkT", bufs=4, name="qkT")
                for half in range(2):
                    chn = 2 * cp + half
                    # fp8 transpose mode requires output element step 2,
                    # so pt is written (and read back) at stride 2
                    pt = ps.tile([128, 768], FP8, tag="tr", bufs=2)
                    for j in range(3):
                        nc.tensor.transpose(
                            _rawap(pt, [[768, 128], [2, 128]], j * 256),
                            qkdw[j][:, chn * 128:(chn + 1) * 128], idb)
                    nc.vector.tensor_copy(qt[:, half * 384:(half + 1) * 384],
                                          _rawap(pt, [[768, 128], [2, 384]],
                                                 0))
                cpi = b * (NCHK // 2) + cp
                last = NBANDS * (NCHK // 2) - 1
                for h in range(HEADS):
                    lhsT = _rawap(qt, [[768, 128], [384, 2], [1, 48]],
                                  h * 48)
                    rhs = _rawap(qt, [[768, 128], [384, 2], [1, 48]],
                                 192 + h * 48)
                    nc.tensor.matmul(
                        gram_ps[0:48, h * 48:(h + 1) * 48], lhsT, rhs,
                        start=(cpi == 0), stop=(cpi == last),
                        perf_mode=mybir.MatmulPerfMode.DoubleRow)

        for step in range(NBANDS + 2):
            if step < NBANDS:
                convA(step)
            if 1 <= step <= NBANDS:
                dwA(step - 1)
            if step == NBANDS:
                convV(0, NVT)
            if step >= 2:
                gramA(step - 2)

        # ---------------- AllReduce of gram + sumsq ----------------
        for j in range(3):
            nc.vector.reduce_sum(ssq[j], ssa[j], axis=AX.X)
        gram_sb = sb.tile([48, 192], F32, tag="gram_sb")
        nc.scalar.copy(gram_sb, gram_ps)
        ar_in = dp.tile([AR_N], F32, tag="ar_in")
        ar_out = dp.tile([AR_N], F32, tag="ar_out")
        nc.gpsimd.dma_start(ar_in[0:9216], gram_sb)
        nc.gpsimd.dma_start(ar_in[9216:9344], ssq[0])
        nc.gpsimd.dma_start(ar_in[9344:9408], ssq[1][0:64, :])
        nc.gpsimd.dma_start(ar_in[9408:9472], ssq[1][64:128, :])
        nc.gpsimd.dma_start(ar_in[9472:9600], ssq[2])
        nc.gpsimd.collective_compute(
            "AllReduce", ALU.add,
            replica_groups=[[0, 1], [2, 3], [4, 5], [6, 7]],
            ins=[ar_in.opt()], outs=[ar_out.opt()])

        # ---------------- Phase B: v path (overlaps AllReduce) ------
        # replicate cv1 rows shifted one image row into partitions 64..127
        for r0, r1 in ((0, 16), (16, 32), (32, 48), (48, 64)):
            nc.sync.dma_start(cv1d[64:128, r0 * RP:r1 * RP],
                              cv1d[0:64, (r0 + 1) * RP:(r1 + 1) * RP])
        # rows 64..65 of the replica are only touched under zero weights,
        # but must hold valid numbers (0 * NaN = NaN in the PE)
        nc.sync.dma_start(cv1d[64:128, 64 * RP:66 * RP],
                          cv1d[0:64, 0:2 * RP])

        def dwV(b, engs=None):
            for s in range(NSUB):
                T = 4 * b + s
                pd = ps.tile([128, 512], F32, tag="dw", bufs=2)
                for ti, (dy, dx) in enumerate(TAPS):
                    rhs = cvp0_3[:, 4 * T + dy: 4 * T + dy + 4,
                                 7 + dx: 7 + dx + W]
                    nc.tensor.matmul(
                        pd, dv0[:, ti * 128:(ti + 1) * 128],
                        rhs, start=(ti == 0), stop=(ti == 8))
                e0, e1 = engs if engs else (
                    (nc.scalar, nc.vector) if s % 2 == 0
                    else (nc.vector, nc.scalar))
                _copy(e0, vdw0[:, T * 512:(T + 1) * 512], pd)
                pd = ps.tile([128, 512], F32, tag="dw", bufs=2)
                for dx in range(3):   # dy 0+1 via packed K=128
                    rhs = cv1d_3[:, 4 * T: 4 * T + 4, 7 + dx: 7 + dx + W]
                    nc.tensor.matmul(
                        pd, dv1pk[:, dx * 128:(dx + 1) * 128],
                        rhs, start=(dx == 0), stop=False)
                for dx in range(3):   # dy=2 singles (zero-padded K=128)
                    rhs = cv1d_3[:, 4 * T + 2: 4 * T + 2 + 4,
                                 7 + dx: 7 + dx + W]
                    nc.tensor.matmul(
                        pd, dv1s[:, dx * 128:(dx + 1) * 128],
                        rhs, start=False, stop=(dx == 2))
                _copy(e1, vdw1[:, T * 512:(T + 1) * 512], pd)

        # memsets for the softmax scratch tiles have no dependencies —
        # hoist them before the dwV drains fill the vector queue
        ones48 = sb.tile([1, 48], F32, tag="ones48")
        nc.vector.memset(ones48, 1.0)
        F0 = sb.tile([128, 192], BF16, tag="E0")
        F1 = sb.tile([64, 192], BF16, tag="E1")
        nc.vector.memset(F0, 0.0)
        nc.vector.memset(F1, 0.0)

        dwV(0)
        dwV(1)

        # ---------------- softmax (after AllReduce) ----------------
        # Emission is split across the dwV bands so each engine's
        # in-order queue reaches the serial chain only when its inputs
        # are ready: the AR-result DMAs (sync queue, which is idle) land
        # after band 1; all vector/scalar compute and the lone K=1
        # broadcast matmul land after band 2, by which point the AR has
        # long completed, and the chain hides under band 3's PE work.
        gram_r = sb.tile([48, 192], F32, tag="gram_r")
        nc.sync.dma_start(gram_r, ar_out[0:9216])
        qss = sb.tile([48, 4], F32, tag="qss")
        nc.sync.dma_start(qss, ar_out[9216:9408].rearrange("(h i) -> i h", i=48))
        kssT = sb.tile([1, 192], F32, tag="kssT")
        nc.sync.dma_start(kssT, ar_out[9408:9600])

        dwV(2)

        rq = sb.tile([48, 4], F32, tag="rq")
        nc.vector.tensor_scalar_max(qss, qss, 1e-24)
        nc.vector.reciprocal(rq, qss)
        nc.scalar.sqrt(rq, rq)
        nc.vector.tensor_mul(rq, rq, tb)
        rkT = sb.tile([1, 192], F32, tag="rkT")
        nc.vector.tensor_scalar_max(kssT, kssT, 1e-24)
        nc.vector.reciprocal(rkT, kssT)
        nc.scalar.sqrt(rkT, rkT)
        # rk broadcast via a single K=1 matmul (PE reaches this well
        # after the AR result landed, so no stall)
        prb = ps.tile([48, 192], F32, tag="conv", bufs=3)
        nc.tensor.matmul(prb, ones48, rkT, start=True, stop=True)
        rkb = sb.tile([48, 192], F32, tag="rkb")
        nc.scalar.copy(rkb, prb)
        g2 = sb.tile([48, 192], F32, tag="g2")
        nc.vector.tensor_mul(g2, gram_r, rkb)
        e = sb.tile([48, 192], F32, tag="e")
        rs = sb.tile([48, 4], F32, tag="rs")
        rc = sb.tile([48, 4], F32, tag="rc")
        for h in range(HEADS):
            hb = slice(h * 48, (h + 1) * 48)
            m = sb.tile([48, 1], F32, tag="mx", bufs=4)
            nc.vector.reduce_max(m, g2[:, hb], axis=AX.X)
            nc.vector.tensor_mul(m, m, rq[:, h:h + 1])
            nc.vector.tensor_scalar_mul(m, m, -1.0)
            nc.scalar.activation(e[:, hb], g2[:, hb], AF.Exp,
                                 bias=m, scale=rq[:, h:h + 1],
                                 accum_out=rs[:, h:h + 1])
        nc.vector.reciprocal(rc, rs)
        e2 = sb.tile([48, 192], F32, tag="e2")
        for h in range(HEADS):
            hb = slice(h * 48, (h + 1) * 48)
            nc.scalar.mul(e2[:, hb], e[:, hb], rc[:, h:h + 1])
        # block-diagonal attn (untransposed): F[c, d] tiles
        nc.gpsimd.dma_start(F0[0:48, 0:48], e2[:, 0:48])
        nc.gpsimd.dma_start(F0[48:96, 48:96], e2[:, 48:96])
        nc.gpsimd.dma_start(F0[96:128, 96:144], e2[0:32, 96:144])
        nc.gpsimd.dma_start(F1[0:16, 96:144], e2[32:48, 96:144])
        nc.gpsimd.dma_start(F1[16:64, 144:192], e2[:, 144:192])

        dwV(3)

        # M[d, o] = sum_c attn[c, d] * Wp[o, c]  (fuses attn@v with proj)
        # stored zero-padded to 256 columns so both ytile lhsT slices are
        # 128 columns wide (keeps FWL enabled).
        pm0 = ps.tile([128, 192], F32, tag="tr", bufs=2)
        pm1 = ps.tile([64, 192], F32, tag="tr", bufs=2)
        for mi, (m0, msz, pm) in enumerate(((0, 128, pm0), (128, 64, pm1))):
            nc.tensor.matmul(pm[0:msz, :], F0[:, m0:m0 + msz], wpT0,
                             start=True, stop=False)
            nc.tensor.matmul(pm[0:msz, :], F1[:, m0:m0 + msz], wpT1,
                             start=False, stop=True)
        Mb0 = sb.tile([128, 256], BF16, tag="Mb0")
        Mb1 = sb.tile([128, 256], BF16, tag="Mb1")
        nc.vector.memset(Mb0[:, 192:256], 0.0)
        nc.vector.memset(Mb1, 0.0)
        nc.scalar.copy(Mb0[:, 0:192], pm0)
        nc.scalar.copy(Mb1[0:64, 0:192], pm1[0:64, :])

        # ---------------- y = M^T @ vdw (single fused stage) --------
        NT = OTOK // 512

        def ytile(nt):
            nsl = slice(nt * 512, (nt + 1) * 512)
            rv0 = _rawap(vdw0, [[OTOK, 128], [128, 4], [1, 128]], nt * 512)
            rv1 = _rawap(vdw1, [[OTOK, 128], [128, 4], [1, 128]], nt * 512)
            py = [ps.tile([128, 512], F32, tag="conv", bufs=3, name="py0"),
                  ps.tile([128, 512], F32, tag="conv", bufs=3, name="py1")]
            for mi, m0 in enumerate((0, 128)):
                nc.tensor.matmul(py[mi], Mb0[:, m0:m0 + 128],
                                 rv0, start=True, stop=False)
                nc.tensor.matmul(py[mi], Mb1[:, m0:m0 + 128],
                                 rv1, start=False, stop=True)
            yb = [sb.tile([128, 512], BF16, tag="yb0", bufs=4, name="yb0"),
                  sb.tile([64, 512], BF16, tag="yb1", bufs=4, name="yb1")]
            _copy(nc.vector, yb[0], py[0])
            _copy(nc.scalar, yb[1], py[1][0:64, :])
            nc.sync.dma_start(out_d[0:128, nsl], yb[0])
            nc.scalar.dma_start(out_d[128:192, nsl], yb[1])

        for nt in range(NT):
            ytile(nt)


_BUILT = None


def _get_built():
    global _BUILT
    if _BUILT is None:
        nc = bacc.Bacc("TRN2", target_bir_lowering=False, debug=False,
                       num_devices=8, num_swdge_queues=4)
        io = {
            "dqk8p": nc.dram_tensor("dqk8p", [128, 9 * 256], FP8,
                                    kind="ExternalInput"),
            "dqk8d": nc.dram_tensor("dqk8d", [128, 3 * 256], FP8,
                                    kind="ExternalInput"),
            "dqk8s": nc.dram_tensor("dqk8s", [128, 3 * 128], FP8,
                                    kind="ExternalInput"),
            "x8": nc.dram_tensor("x8", [96, 2 * SH_ROWS * W], FP8,
                                 kind="ExternalInput"),
            "wq8": nc.dram_tensor("wq8", [96, 2 * 384], FP8,
                                  kind="ExternalInput"),
            "x": nc.dram_tensor("x", [192, SH_ROWS, W], BF16,
                                kind="ExternalInput"),
            "wqkvT": nc.dram_tensor("wqkvT", [256, 640], BF16,
                                    kind="ExternalInput"),
            "dv0": nc.dram_tensor("dv0", [128, 9 * 128], BF16,
                                  kind="ExternalInput"),
            "dv1pk": nc.dram_tensor("dv1pk", [128, 3 * 128], BF16,
                                    kind="ExternalInput"),
            "dv1s": nc.dram_tensor("dv1s", [128, 3 * 128], BF16,
                                   kind="ExternalInput"),
            "wprojT": nc.dram_tensor("wprojT", [C, C], BF16,
                                     kind="ExternalInput"),
            "idf": nc.dram_tensor("idf", [128, 128], F32,
                                  kind="ExternalInput"),
            "tempb": nc.dram_tensor("tempb", [48, 4], F32,
                                    kind="ExternalInput"),
            "out": nc.dram_tensor("out", [C, OUT_ROWS, W], BF16,
                                  kind="ExternalOutput"),
        }
        with TileContext(nc) as tc:
            _program(nc, tc, io)
        nc.compile()
        _BUILT = nc
    return _BUILT


def _make_in_maps(x, w_qkv, w_dw, w_proj, temperature):
    wqkvT = np.zeros((256, 640), np.float32)
    wqkvT[0:C, 0:576] = w_qkv[:, :, 0, 0].T
    wqkvT = wqkvT.astype(BF16NP)
    wd = w_dw[:, 0, :, :].astype(np.float32)           # [576, 3, 3]
    dv0 = np.zeros((9, 128, 128), np.float32)
    for t, (dy, dx) in enumerate(TAPS):
        np.fill_diagonal(dv0[t], wd[384:512, dy, dx])
    dv0 = np.ascontiguousarray(
        dv0.transpose(1, 0, 2).reshape(128, 9 * 128)).astype(BF16NP)
    dv1pk = np.zeros((3, 128, 128), np.float32)
    for dx in range(3):
        np.fill_diagonal(dv1pk[dx, 0:64, 0:64], wd[512:576, 0, dx])
        np.fill_diagonal(dv1pk[dx, 64:128, 0:64], wd[512:576, 1, dx])
    dv1pk = np.ascontiguousarray(
        dv1pk.transpose(1, 0, 2).reshape(128, 3 * 128)).astype(BF16NP)
    dv1s = np.zeros((3, 128, 128), np.float32)
    for dx in range(3):
        np.fill_diagonal(dv1s[dx, 0:64, 0:64], wd[512:576, 2, dx])
    dv1s = np.ascontiguousarray(
        dv1s.transpose(1, 0, 2).reshape(128, 3 * 128)).astype(BF16NP)
    # fp8 DoubleRow diag weights for the q/k depthwise conv:
    # dqk8p pairs (dy0,dy1) per dx; dqk8d pairs (dy2,dx0)+(dy2,dx1) via
    # the in-SBUF replica; dqk8s is the lone (dy2,dx2) tap
    dqk8p = np.zeros((128, 9, 2, 128), np.float32)
    dqk8d = np.zeros((128, 3, 2, 128), np.float32)
    dqk8s = np.zeros((128, 3, 128), np.float32)
    for j in range(3):
        blk = wd[j * 128:(j + 1) * 128]                # [128, 3, 3]
        for dx in range(3):
            for i in range(2):
                np.fill_diagonal(dqk8p[:, j * 3 + dx, i, :], blk[:, i, dx])
        for i in range(2):
            np.fill_diagonal(dqk8d[:, j, i, :], blk[:, 2, i])
        np.fill_diagonal(dqk8s[:, j, :], blk[:, 2, 2])
    dqk8p = dqk8p.reshape(128, 9 * 256).astype(FP8NP)
    dqk8d = dqk8d.reshape(128, 3 * 256).astype(FP8NP)
    dqk8s = dqk8s.reshape(128, 3 * 128).astype(FP8NP)
    # fp8 paired-channel conv weights: wq8[k, i, m] = wqkvT[k + 96 i, m<384]
    wq8 = np.stack([w_qkv[:, :, 0, 0].T[:96, :384],
                    w_qkv[:, :, 0, 0].T[96:192, :384]], axis=1)
    wq8 = np.ascontiguousarray(wq8.astype(np.float32)).reshape(
        96, 2 * 384).astype(FP8NP)

    wprojT = np.ascontiguousarray(w_proj[:, :, 0, 0].T).astype(BF16NP)
    idf = np.eye(128, dtype=np.float32)
    tempb = np.ascontiguousarray(np.broadcast_to(
        np.asarray(temperature, np.float32).reshape(1, HEADS), (48, HEADS)))
    in_maps = []
    for core in range(8):
        b, s = core // 2, core % 2
        xs = np.zeros((C, SH_ROWS, W), BF16NP)
        r0 = s * OUT_ROWS - 1
        lo, hi = max(r0, 0), min(r0 + SH_ROWS, 128)
        xs[:, lo - r0: hi - r0, :] = x[b, :, lo:hi, :].astype(BF16NP)
        xf = xs.reshape(C, SH_ROWS * W).astype(np.float32)
        # channel pairs (c, c+96) interleaved per image row so the
        # DoubleRow pair step is 128 and inner runs stay 128 elements
        x8 = xf.reshape(2, 96, SH_ROWS, W).transpose(1, 2, 0, 3)
        x8 = np.ascontiguousarray(x8).reshape(96, 2 * NTOK).astype(FP8NP)
        in_maps.append({
            "x": xs, "wqkvT": wqkvT, "dqk8p": dqk8p, "dqk8d": dqk8d,
            "dqk8s": dqk8s,
            "x8": x8, "wq8": wq8, "dv0": dv0, "dv1pk": dv1pk, "dv1s": dv1s,
            "wprojT": wprojT, "idf": idf, "tempb": tempb,
        })
    return in_maps


def kernel(x, w_qkv, w_dw, w_proj, temperature):
    x = np.asarray(x, np.float32)
    nc = _get_built()
    in_maps = _make_in_maps(np.asarray(x, np.float32),
                            np.asarray(w_qkv, np.float32),
                            np.asarray(w_dw, np.float32),
                            np.asarray(w_proj, np.float32),
                            np.asarray(temperature, np.float32))
    res = run_bass_kernel_spmd(nc, in_maps, core_ids=list(range(8)))
    y = np.empty((4, C, 128, W), np.float32)
    for core in range(8):
        b, s = core // 2, core % 2
        y[b, :, s * OUT_ROWS:(s + 1) * OUT_ROWS, :] = np.asarray(
            res.results[core]["out"], np.float32)
    return y



# revision 46
# speedup vs baseline: 1.1762x; 1.1762x over previous
"""Trainium2 Bass kernel for Restormer-style channel attention (XCA).

Reference computation (B=4, C=192, H=W=128, HEADS=4, ch=48):
  qkv = dwconv3x3(conv1x1(x, w_qkv), w_dw)       # [B, 576, H, W]
  q, k, v = split(qkv); per head: L2-normalize q, k over tokens
  attn = softmax(q @ k^T * temp)                 # [B, 4, 48, 48]
  out = conv1x1((attn @ v).reshape, w_proj)

Sharding: 8 cores = 4 batches x 2 row-halves (64 image rows each, +1 halo
row each side).  Per-core gram matrices + channel sumsqs are AllReduce'd
between the two cores of a batch (38 KB), overlapped with the v path.

Depthwise conv runs on the TensorEngine as diagonal matmuls with PSUM
tap-accumulation, reading shifted views of zero-padded pitch-144 buffers.
The q/k path uses fp8 DoubleRow for conv1x1 (channel pairs interleaved
per image row so the DR pair step is 128 and inner runs stay 128 long)
and for the depthwise conv in FIVE passes per 512-token subtile: 3
dy-pairs at step RP=144, one (dy2,dx0)+(dy2,dx1) pair whose second
group reads a row replica placed at offset 18*RP+15 so the pair step
(2320) is 16-aligned, and one (dy2,dx2) single.  qkdw is stored fp8
(the gram consumes it as fp8 anyway); its PE transposes write PSUM at
element step 2 as fp8 transpose mode requires.
The v path stays bf16 (output noise propagates ~1:1); its conv1x1 runs
once over the 66-row shard into global padded buffers, and every K=64
matmul is zero-padded to K=128 (the PE pays ~100 ns on any matmul
adjacent to a 64-row-config one); x1's dead upper half is an SBUF copy
of real data under zero weights instead of 1 MB of HBM zeros.
attn@v and the output projection are fused into a single matmul stage
via M = blockdiag(attn) @ WprojT computed on device.  PSUM drains
alternate between ScalarE and VectorE; the post-AllReduce softmax chain
is emitted piecewise between dwV bands (AR DMAs on the idle sync queue
after band 1, all vector/scalar compute plus a single K=1 broadcast
matmul for the k-norm after band 2) so the in-order engine queues never
stall the PE on the AllReduce; the temperature broadcast is precomputed
on the host.
"""

import sys

for p in ("/opt/trn_rl_repo", "/root/.axon_site/_ro/trn_rl_repo"):
    if p not in sys.path:
        sys.path.insert(0, p)

import numpy as np
import ml_dtypes
import bass_rust

BF16NP = ml_dtypes.bfloat16

import concourse.bass as bass
import concourse.bacc as bacc
import concourse.mybir as mybir
from concourse.tile import TileContext
from concourse.bass_utils import run_bass_kernel_spmd

F32 = mybir.dt.float32
F32R = mybir.dt.float32r
BF16 = mybir.dt.bfloat16
FP8 = mybir.dt.float8e4
FP8NP = mybir.dt.np(FP8)
AF = mybir.ActivationFunctionType
AX = mybir.AxisListType
ALU = mybir.AluOpType

C = 192
HEADS = 4
CH = 48
W = 128
SH_ROWS = 66          # shard rows incl. 1 halo row each side
OUT_ROWS = 64
BAND = 16             # output rows per band
NBANDS = OUT_ROWS // BAND
RP = 144              # padded row pitch (16-aligned for fp8 DoubleRow)
NTOK = SH_ROWS * W    # 8448 tokens incl halo rows
OTOK = OUT_ROWS * W   # 8192 output tokens
TAPS = [(t // 3, t % 3) for t in range(9)]
AR_N = 9600           # 48*192 gram + 192 qss + 192 kss
# q/k band buffer: 18 padded rows + a replica of rows 2..17 placed at
# offset 18*RP+15 so the (dy2,dx0)/(dy2,dx1) DoubleRow pair step
# (18*RP+15+1-2*RP = 2320) is 16-aligned.
PB_REP = 18 * RP + 15         # replica offset inside pb
PB_W = PB_REP + 16 * RP       # pb tile width (4911)
PB_STEP = PB_REP + 1 - 2 * RP  # 2320, the dy2 dx-pair step


def r32(ap):
    return ap.bitcast(F32R)


def _rawap(ap, pairs, offset):
    c = ap.copy()
    c.ap = bass_rust.VecI64Pair(pairs)
    c.offset = offset
    return c


def _copy(eng, dst, src):
    if getattr(eng, "engine", None) == mybir.EngineType.Activation:
        return eng.copy(dst, src)
    return eng.tensor_copy(dst, src)


def _program(nc, tc, io):
    x_d = io["x"].ap().rearrange("c h w -> c (h w)")          # [192, 8448]
    wqkvT_d = io["wqkvT"].ap()                                # [256, 640]
    dqk8p_d = io["dqk8p"].ap()                                # [128, 9*2*128]
    dqk8d_d = io["dqk8d"].ap()                                # [128, 3*2*128]
    dqk8s_d = io["dqk8s"].ap()                                # [128, 3*128]
    x8_d = io["x8"].ap()                                      # [96, 2*8448]
    wq8_d = io["wq8"].ap()                                    # [96, 2*384]
    dv0_d = io["dv0"].ap()                                    # [128, 9*128]
    dv1pk_d = io["dv1pk"].ap()                                # [128, 3*128]
    dv1s_d = io["dv1s"].ap()                                  # [128, 3*128]
    wprojT_d = io["wprojT"].ap()                              # [192, 192]
    idf_d = io["idf"].ap()                                    # [128, 128]
    tempb_d = io["tempb"].ap()                                # [48, 4]
    out_d = io["out"].ap().rearrange("c h w -> c (h w)")      # [192, 8192]

    CTOK = (BAND + 2) * W            # conv tokens per band (2304)
    NSUB = BAND * W // 512           # dw 512-token subtiles per band (4)
    NCHK = BAND                      # 128-token chunks per band (16)

    with (
        tc.tile_pool(name="sb", bufs=1) as sb,
        tc.tile_pool(name="ps", bufs=1, space="PSUM") as ps,
        tc.tile_pool(name="dram", bufs=1, space="DRAM") as dp,
    ):
        # ---------------- persistent SBUF loads ----------------
        # fp8 x for the q/k conv: pairs (c, c+96) interleaved per image
        # row, so the DoubleRow pair step is 128 (16-aligned) and the
        # moving AP keeps 128-element inner runs (the fast PE rhs shape).
        x8 = sb.tile([96, 2 * NTOK], FP8, tag="x8")
        NXC = 16
        csz = NTOK // NXC
        qs = (nc.sync, nc.scalar)
        wq8 = sb.tile([96, 2 * 384], FP8, tag="wq8")
        nc.scalar.dma_start(wq8, wq8_d)
        # first 9 small chunks cover band 0 quickly, then the q/k dw
        # weights (needed by dwA(0) shortly after), then the rest
        bounds = [0, 256, 512, 768, 1024, 1280, 1536, 1792, 2048, 2304]
        step = (NTOK - 2304) // 8
        bounds += [2304 + step * i for i in range(1, 8)] + [NTOK]
        for j in range(len(bounds) - 1):
            a, bnd = bounds[j], bounds[j + 1]
            qs[j % 2].dma_start(x8[:, 2 * a:2 * bnd], x8_d[:, 2 * a:2 * bnd])
        # warm-up runs on a memset dummy so the PE starts before any DMA
        wdum = sb.tile([128, 128], BF16, tag="wdum")
        nc.vector.memset(wdum, 0.0)
        for wi in range(16):
            pw = ps.tile([128, 128], F32, tag="dw", bufs=2, name="pw")
            for wj in range(4):
                nc.tensor.matmul(pw, wdum, wdum, start=(wj == 0),
                                 stop=(wj == 3))
        idf = sb.tile([128, 128], F32, tag="idf")
        nc.gpsimd.dma_start(idf, idf_d)
        idb = sb.tile([128, 128], FP8, tag="idb")
        nc.scalar.copy(idb, idf)
        idb_box = [idb]
        tb = sb.tile([48, 4], F32, tag="tb")
        nc.sync.dma_start(tb, tempb_d)
        # small weight tensors first on the SWDGE queue (the q/k dw needs
        # dqk8p/dqk8d/dqk8s before band 0 finishes its conv)
        dqk8p = sb.tile([128, 9 * 256], FP8, tag="dqk8p")
        nc.gpsimd.dma_start(dqk8p, dqk8p_d)
        dqk8d = sb.tile([128, 3 * 256], FP8, tag="dqk8d")
        nc.gpsimd.dma_start(dqk8d, dqk8d_d)
        dqk8s = sb.tile([128, 3 * 128], FP8, tag="dqk8s")
        nc.gpsimd.dma_start(dqk8s, dqk8s_d)
        wqkvT0 = sb.tile([128, 640], BF16, tag="wqkvT0")
        wqkvT1 = sb.tile([128, 640], BF16, tag="wqkvT1")
        nc.gpsimd.dma_start(wqkvT0, wqkvT_d[0:128, :])
        nc.gpsimd.dma_start(wqkvT1, wqkvT_d[128:256, :])
        dv0 = sb.tile([128, 9 * 128], BF16, tag="dv0")
        nc.gpsimd.dma_start(dv0, dv0_d)
        dv1pk = sb.tile([128, 3 * 128], BF16, tag="dv1pk")
        nc.gpsimd.dma_start(dv1pk, dv1pk_d)
        dv1s = sb.tile([128, 3 * 128], BF16, tag="dv1s")
        nc.gpsimd.dma_start(dv1s, dv1s_d)
        wpT0 = sb.tile([128, 192], BF16, tag="wpT0")
        wpT1 = sb.tile([64, 192], BF16, tag="wpT1")
        nc.gpsimd.dma_start(wpT0, wprojT_d[0:128, :])
        nc.gpsimd.dma_start(wpT1, wprojT_d[128:192, :])
        # bf16 x for the v path; x1's upper 64 partitions are filled with
        # a copy of its lower half (their wqkvT rows are zero) so both
        # conv K-blocks are full 128-row matmuls (the PE pays a ~100 ns
        # penalty on every matmul adjacent to a 64-row one) without
        # DMAing 1 MB of zeros from HBM.  The chunk loads are emitted
        # inside convA(b) (4 per band) so the early DMA engines belong
        # to the latency-critical x8 load.
        x0 = sb.tile([128, NTOK], BF16, tag="x0")
        x1 = sb.tile([128, NTOK], BF16, tag="x1")

        def loadx(j):
            nc.gpsimd.dma_start(x0[:, j * csz:(j + 1) * csz],
                                x_d[0:128, j * csz:(j + 1) * csz])
            nc.gpsimd.dma_start(x1[0:64, j * csz:(j + 1) * csz],
                                x_d[128:192, j * csz:(j + 1) * csz])
            nc.sync.dma_start(x1[64:128, j * csz:(j + 1) * csz],
                              x1[0:64, j * csz:(j + 1) * csz])

        ssqp = [sb.tile([128, 16], F32, tag=f"ssqp{j}", name=f"ssqp{j}")
                for j in range(3)]
        ssq = [sb.tile([128, 1], F32, tag=f"ssq{j}", name=f"ssq{j}")
               for j in range(3)]

        gram_ps = ps.tile([48, 192], F32, tag="gram")

        # v-path global buffers (allocated early; first conv tiles are
        # emitted inside the phase-A pipeline to fill the gram tail gap)
        vdw0 = sb.tile([128, OTOK], BF16, tag="vdw0")
        vdw1 = sb.tile([128, OTOK], BF16, tag="vdw1")
        cvp0 = sb.tile([128, SH_ROWS * RP], BF16, tag="cvp0")
        cv1d = sb.tile([128, SH_ROWS * RP], BF16, tag="cv1d")
        cvp0_3 = cvp0.rearrange("p (r w) -> p r w", w=RP)
        cv1d_3 = cv1d.rearrange("p (r w) -> p r w", w=RP)
        nc.vector.memset(cvp0_3[:, :, 7:8], 0.0)
        nc.vector.memset(cvp0_3[:, :, 8 + W:9 + W], 0.0)
        nc.vector.memset(cv1d_3[0:64, :, 7:8], 0.0)
        nc.vector.memset(cv1d_3[0:64, :, 8 + W:9 + W], 0.0)

        NVT = (NTOK + 511) // 512     # 17 conv tiles over 66 rows

        def convV(t0, t1):
            for t in range(t0, t1):
                n0 = t * 512
                nsz = min(512, NTOK - n0)
                for j, m0 in enumerate((384, 512)):
                    pc = ps.tile([128, 512], F32, tag="conv", bufs=3)
                    r0v = _rawap(x0, [[NTOK, 128], [128, nsz // 128],
                                      [1, 128]], n0)
                    r1v = _rawap(x1, [[NTOK, 128], [128, nsz // 128],
                                      [1, 128]], n0)
                    nc.tensor.matmul(pc[:, 0:nsz], wqkvT0[:, m0:m0 + 128],
                                     r0v, start=True, stop=False)
                    nc.tensor.matmul(pc[:, 0:nsz], wqkvT1[:, m0:m0 + 128],
                                     r1v, start=False, stop=True)
                    r0 = n0 // W
                    dst3 = cvp0_3 if j == 0 else cv1d_3
                    msz = 128 if j == 0 else 64
                    eng = nc.scalar if (j + t) % 2 == 0 else nc.vector
                    _copy(eng,
                          dst3[0:msz, r0:r0 + nsz // W, 8:8 + W],
                          pc[0:msz, 0:nsz].rearrange("p (r w) -> p r w",
                                                     w=W))

        CONV_NT = tuple((i * 512, min(512, CTOK - i * 512))
                        for i in range((CTOK + 511) // 512))

        # ---------------- Phase A: q/k path, software-pipelined ------
        pb_of, qkdw_of = {}, {}

        wq83 = wq8.rearrange("p (i m) -> p i m", m=384)

        def convA(b):
            for j in range(4 * b, 4 * b + 4):
                loadx(j)
            tok0 = b * BAND * W
            pb = [sb.tile([128, PB_W], FP8, tag=f"pb{j}", bufs=2,
                          name=f"pb{j}") for j in range(3)]
            pb3 = [t[:, 0:18 * RP].rearrange("p (r w) -> p r w", w=RP)
                   for t in pb]
            for j in range(3):
                nc.vector.memset(pb3[j][:, :, 7:8], 0.0)
                nc.vector.memset(pb3[j][:, :, 8 + W:9 + W], 0.0)
            for j in range(3):
                m0 = j * 128
                for n0, nsz in CONV_NT:
                    pc = ps.tile([128, 512], F32, tag="conv", bufs=3)
                    rhs = _rawap(x8, [[2 * NTOK, 96], [128, 2],
                                      [256, nsz // 128], [1, 128]],
                                 2 * (tok0 + n0))
                    nc.tensor.matmul(pc[:, 0:nsz], wq83[:, :, m0:m0 + 128],
                                     rhs, start=True, stop=True,
                                     perf_mode=mybir.MatmulPerfMode.DoubleRow)
                    r0 = n0 // W
                    eng = nc.scalar if (j + n0 // 512) % 2 == 0 else nc.vector
                    _copy(eng,
                          pb3[j][:, r0:r0 + nsz // W, 8:8 + W],
                          pc[:, 0:nsz].rearrange("p (r w) -> p r w", w=W))
            # replicate rows 2..17 at the 16-alignment-fixing offset for
            # the (dy2,dx0)+(dy2,dx1) DoubleRow pair
            for j in range(3):
                (nc.sync, nc.scalar, nc.gpsimd)[j].dma_start(
                    pb[j][:, PB_REP:PB_REP + 16 * RP],
                    pb[j][:, 2 * RP:18 * RP])
            pb_of[b] = pb

        def dwA(b):
            pb = pb_of.pop(b)
            pb3 = [t[:, 0:18 * RP].rearrange("p (r w) -> p r w", w=RP)
                   for t in pb]
            qkdw = [sb.tile([128, BAND * W], FP8, tag=f"qkdw{j}", bufs=2,
                            name=f"qkdw{j}") for j in range(3)]
            for j in range(3):
                for s in range(NSUB):
                    pd = ps.tile([128, 512], F32, tag="dw", bufs=2)
                    for dx in range(3):
                        rhs = _rawap(pb[j],
                                     [[PB_W, 128], [RP, 2], [RP, 4],
                                      [1, W]],
                                     4 * s * RP + 7 + dx)
                        nc.tensor.matmul(
                            pd, dqk8p[:, (j * 3 + dx) * 256:
                                      (j * 3 + dx + 1) * 256]
                            .rearrange("p (i m) -> p i m", m=128),
                            rhs, start=(dx == 0), stop=False,
                            perf_mode=mybir.MatmulPerfMode.DoubleRow)
                    # (dy2,dx0)+(dy2,dx1) in one DoubleRow pass via the
                    # replica region (pair step PB_STEP, 16-aligned)
                    rhs = _rawap(pb[j],
                                 [[PB_W, 128], [PB_STEP, 2], [RP, 4],
                                  [1, W]],
                                 (4 * s + 2) * RP + 7)
                    nc.tensor.matmul(
                        pd, dqk8d[:, j * 256:(j + 1) * 256]
                        .rearrange("p (i m) -> p i m", m=128),
                        rhs, start=False, stop=False,
                        perf_mode=mybir.MatmulPerfMode.DoubleRow)
                    # (dy2,dx2) single
                    rhs = pb3[j][:, 4 * s + 2: 4 * s + 2 + 4,
                                 9: 9 + W]
                    nc.tensor.matmul(
                        pd, dqk8s[:, j * 128:(j + 1) * 128],
                        rhs, start=False, stop=True)
                    eng = nc.scalar if s % 2 == 0 else nc.vector
                    _copy(eng, qkdw[j][:, s * 512:(s + 1) * 512], pd)
                    qs = qkdw[j][:, s * 512:(s + 1) * 512]
                    sq = sb.tile([128, 512], F32, tag="sqs", bufs=2)
                    nc.scalar.activation(sq, qs, AF.Square,
                                         accum_out=ssqp[j][:, 4 * b + s:
                                                           4 * b + s + 1])
            qkdw_of[b] = qkdw

        def gramA(b):
            qkdw = qkdw_of.pop(b)
            idb = idb_box[0]
            for cp in range(NCHK // 2):
                qt = sb.tile([128, 768], FP8, tag="qkT", bufs=4, name="qkT")
                for half in range(2):
                    chn = 2 * cp + half
                    # fp8 transpose mode requires output element step 2,
                    # so pt is written (and read back) at stride 2
                    pt = ps.tile([128, 768], FP8, tag="tr", bufs=2)
                    for j in range(3):
                        nc.tensor.transpose(
                            _rawap(pt, [[768, 128], [2, 128]], j * 256),
                            qkdw[j][:, chn * 128:(chn + 1) * 128], idb)
                    nc.vector.tensor_copy(qt[:, half * 384:(half + 1) * 384],
                                          _rawap(pt, [[768, 128], [2, 384]],
                                                 0))
                cpi = b * (NCHK // 2) + cp
                last = NBANDS * (NCHK // 2) - 1
                for h in range(HEADS):
                    lhsT = _rawap(qt, [[768, 128], [384, 2], [1, 48]],
                                  h * 48)
                    rhs = _rawap(qt, [[768, 128], [384, 2], [1, 48]],
                                 192 + h * 48)
                    nc.tensor.matmul(
                        gram_ps[0:48, h * 48:(h + 1) * 48], lhsT, rhs,
                        start=(cpi == 0), stop=(cpi == last),
                        perf_mode=mybir.MatmulPerfMode.DoubleRow)

        for step in range(NBANDS + 2):
            if step < NBANDS:
                convA(step)
            if 1 <= step <= NBANDS:
                dwA(step - 1)
            if step == NBANDS:
                convV(0, NVT)
            if step >= 2:
                gramA(step - 2)

        # ---------------- AllReduce of gram + sumsq ----------------
        for j in range(3):
            nc.vector.reduce_sum(ssq[j], ssqp[j], axis=AX.X)
        gram_sb = sb.tile([48, 192], F32, tag="gram_sb")
        nc.scalar.copy(gram_sb, gram_ps)
        ar_in = dp.tile([AR_N], F32, tag="ar_in")
        ar_out = dp.tile([AR_N], F32, tag="ar_out")
        nc.gpsimd.dma_start(ar_in[0:9216], gram_sb)
        nc.gpsimd.dma_start(ar_in[9216:9344], ssq[0])
        nc.gpsimd.dma_start(ar_in[9344:9408], ssq[1][0:64, :])
        nc.gpsimd.dma_start(ar_in[9408:9472], ssq[1][64:128, :])
        nc.gpsimd.dma_start(ar_in[9472:9600], ssq[2])
        nc.gpsimd.collective_compute(
            "AllReduce", ALU.add,
            replica_groups=[[0, 1], [2, 3], [4, 5], [6, 7]],
            ins=[ar_in.opt()], outs=[ar_out.opt()])

        # ---------------- Phase B: v path (overlaps AllReduce) ------
        # replicate cv1 rows shifted one image row into partitions 64..127
        for r0, r1 in ((0, 16), (16, 32), (32, 48), (48, 64)):
            nc.sync.dma_start(cv1d[64:128, r0 * RP:r1 * RP],
                              cv1d[0:64, (r0 + 1) * RP:(r1 + 1) * RP])
        # rows 64..65 of the replica are only touched under zero weights,
        # but must hold valid numbers (0 * NaN = NaN in the PE)
        nc.sync.dma_start(cv1d[64:128, 64 * RP:66 * RP],
                          cv1d[0:64, 0:2 * RP])

        def dwV(b, engs=None):
            for s in range(NSUB):
                T = 4 * b + s
                pd = ps.tile([128, 512], F32, tag="dw", bufs=2)
                for ti, (dy, dx) in enumerate(TAPS):
                    rhs = cvp0_3[:, 4 * T + dy: 4 * T + dy + 4,
                                 7 + dx: 7 + dx + W]
                    nc.tensor.matmul(
                        pd, dv0[:, ti * 128:(ti + 1) * 128],
                        rhs, start=(ti == 0), stop=(ti == 8))
                e0, e1 = engs if engs else (
                    (nc.scalar, nc.vector) if s % 2 == 0
                    else (nc.vector, nc.scalar))
                _copy(e0, vdw0[:, T * 512:(T + 1) * 512], pd)
                pd = ps.tile([128, 512], F32, tag="dw", bufs=2)
                for dx in range(3):   # dy 0+1 via packed K=128
                    rhs = cv1d_3[:, 4 * T: 4 * T + 4, 7 + dx: 7 + dx + W]
                    nc.tensor.matmul(
                        pd, dv1pk[:, dx * 128:(dx + 1) * 128],
                        rhs, start=(dx == 0), stop=False)
                for dx in range(3):   # dy=2 singles (zero-padded K=128)
                    rhs = cv1d_3[:, 4 * T + 2: 4 * T + 2 + 4,
                                 7 + dx: 7 + dx + W]
                    nc.tensor.matmul(
                        pd, dv1s[:, dx * 128:(dx + 1) * 128],
                        rhs, start=False, stop=(dx == 2))
                _copy(e1, vdw1[:, T * 512:(T + 1) * 512], pd)

        # memsets for the softmax scratch tiles have no dependencies —
        # hoist them before the dwV drains fill the vector queue
        ones48 = sb.tile([1, 48], F32, tag="ones48")
        nc.vector.memset(ones48, 1.0)
        F0 = sb.tile([128, 192], BF16, tag="E0")
        F1 = sb.tile([64, 192], BF16, tag="E1")
        nc.vector.memset(F0, 0.0)
        nc.vector.memset(F1, 0.0)

        dwV(0)
        dwV(1)

        # ---------------- softmax (after AllReduce) ----------------
        # Emission is split across the dwV bands so each engine's
        # in-order queue reaches the serial chain only when its inputs
        # are ready: the AR-result DMAs (sync queue, which is idle) land
        # after band 1; all vector/scalar compute and the lone K=1
        # broadcast matmul land after band 2, by which point the AR has
        # long completed, and the chain hides under band 3's PE work.
        gram_r = sb.tile([48, 192], F32, tag="gram_r")
        nc.sync.dma_start(gram_r, ar_out[0:9216])
        qss = sb.tile([48, 4], F32, tag="qss")
        nc.sync.dma_start(qss, ar_out[9216:9408].rearrange("(h i) -> i h", i=48))
        kssT = sb.tile([1, 192], F32, tag="kssT")
        nc.sync.dma_start(kssT, ar_out[9408:9600])

        dwV(2)

        rq = sb.tile([48, 4], F32, tag="rq")
        nc.vector.tensor_scalar_max(qss, qss, 1e-24)
        nc.vector.reciprocal(rq, qss)
        nc.scalar.sqrt(rq, rq)
        nc.vector.tensor_mul(rq, rq, tb)
        rkT = sb.tile([1, 192], F32, tag="rkT")
        nc.vector.tensor_scalar_max(kssT, kssT, 1e-24)
        nc.vector.reciprocal(rkT, kssT)
        nc.scalar.sqrt(rkT, rkT)
        # rk broadcast via a single K=1 matmul (PE reaches this well
        # after the AR result landed, so no stall)
        prb = ps.tile([48, 192], F32, tag="conv", bufs=3)
        nc.tensor.matmul(prb, ones48, rkT, start=True, stop=True)
        rkb = sb.tile([48, 192], F32, tag="rkb")
        nc.scalar.copy(rkb, prb)
        g2 = sb.tile([48, 192], F32, tag="g2")
        nc.vector.tensor_mul(g2, gram_r, rkb)
        e = sb.tile([48, 192], F32, tag="e")
        rs = sb.tile([48, 4], F32, tag="rs")
        rc = sb.tile([48, 4], F32, tag="rc")
        for h in range(HEADS):
            hb = slice(h * 48, (h + 1) * 48)
            m = sb.tile([48, 1], F32, tag="mx", bufs=4)
            nc.vector.reduce_max(m, g2[:, hb], axis=AX.X)
            nc.vector.tensor_mul(m, m, rq[:, h:h + 1])
            nc.vector.tensor_scalar_mul(m, m, -1.0)
            nc.scalar.activation(e[:, hb], g2[:, hb], AF.Exp,
                                 bias=m, scale=rq[:, h:h + 1],
                                 accum_out=rs[:, h:h + 1])
        nc.vector.reciprocal(rc, rs)
        e2 = sb.tile([48, 192], F32, tag="e2")
        for h in range(HEADS):
            hb = slice(h * 48, (h + 1) * 48)
            nc.scalar.mul(e2[:, hb], e[:, hb], rc[:, h:h + 1])
        # block-diagonal attn (untransposed): F[c, d] tiles
        nc.gpsimd.dma_start(F0[0:48, 0:48], e2[:, 0:48])
        nc.gpsimd.dma_start(F0[48:96, 48:96], e2[:, 48:96])
        nc.gpsimd.dma_start(F0[96:128, 96:144], e2[0:32, 96:144])
        nc.gpsimd.dma_start(F1[0:16, 96:144], e2[32:48, 96:144])
        nc.gpsimd.dma_start(F1[16:64, 144:192], e2[:, 144:192])

        dwV(3)

        # M[d, o] = sum_c attn[c, d] * Wp[o, c]  (fuses attn@v with proj)
        # stored zero-padded to 256 columns so both ytile lhsT slices are
        # 128 columns wide (keeps FWL enabled).
        pm0 = ps.tile([128, 192], F32, tag="tr", bufs=2)
        pm1 = ps.tile([64, 192], F32, tag="tr", bufs=2)
        for mi, (m0, msz, pm) in enumerate(((0, 128, pm0), (128, 64, pm1))):
            nc.tensor.matmul(pm[0:msz, :], F0[:, m0:m0 + msz], wpT0,
                             start=True, stop=False)
            nc.tensor.matmul(pm[0:msz, :], F1[:, m0:m0 + msz], wpT1,
                             start=False, stop=True)
        Mb0 = sb.tile([128, 256], BF16, tag="Mb0")
        Mb1 = sb.tile([128, 256], BF16, tag="Mb1")
        nc.vector.memset(Mb0[:, 192:256], 0.0)
        nc.vector.memset(Mb1, 0.0)
        nc.scalar.copy(Mb0[:, 0:192], pm0)
        nc.scalar.copy(Mb1[0:64, 0:192], pm1[0:64, :])

        # ---------------- y = M^T @ vdw (single fused stage) --------
        NT = OTOK // 512

        def ytile(nt):
            nsl = slice(nt * 512, (nt + 1) * 512)
            rv0 = _rawap(vdw0, [[OTOK, 128], [128, 4], [1, 128]], nt * 512)
            rv1 = _rawap(vdw1, [[OTOK, 128], [128, 4], [1, 128]], nt * 512)
            py = [ps.tile([128, 512], F32, tag="conv", bufs=3, name="py0"),
                  ps.tile([128, 512], F32, tag="conv", bufs=3, name="py1")]
            for mi, m0 in enumerate((0, 128)):
                nc.tensor.matmul(py[mi], Mb0[:, m0:m0 + 128],
                                 rv0, start=True, stop=False)
                nc.tensor.matmul(py[mi], Mb1[:, m0:m0 + 128],
                                 rv1, start=False, stop=True)
            yb = [sb.tile([128, 512], BF16, tag="yb0", bufs=4, name="yb0"),
                  sb.tile([64, 512], BF16, tag="yb1", bufs=4, name="yb1")]
            _copy(nc.vector, yb[0], py[0])
            _copy(nc.scalar, yb[1], py[1][0:64, :])
            nc.sync.dma_start(out_d[0:128, nsl], yb[0])
            nc.scalar.dma_start(out_d[128:192, nsl], yb[1])

        for nt in range(NT):
            ytile(nt)


_BUILT = None


def _get_built():
    global _BUILT
    if _BUILT is None:
        nc = bacc.Bacc("TRN2", target_bir_lowering=False, debug=False,
                       num_devices=8, num_swdge_queues=4)
        io = {
            "dqk8p": nc.dram_tensor("dqk8p", [128, 9 * 256], FP8,
                                    kind="ExternalInput"),
            "dqk8d": nc.dram_tensor("dqk8d", [128, 3 * 256], FP8,
                                    kind="ExternalInput"),
            "dqk8s": nc.dram_tensor("dqk8s", [128, 3 * 128], FP8,
                                    kind="ExternalInput"),
            "x8": nc.dram_tensor("x8", [96, 2 * SH_ROWS * W], FP8,
                                 kind="ExternalInput"),
            "wq8": nc.dram_tensor("wq8", [96, 2 * 384], FP8,
                                  kind="ExternalInput"),
            "x": nc.dram_tensor("x", [192, SH_ROWS, W], BF16,
                                kind="ExternalInput"),
            "wqkvT": nc.dram_tensor("wqkvT", [256, 640], BF16,
                                    kind="ExternalInput"),
            "dv0": nc.dram_tensor("dv0", [128, 9 * 128], BF16,
                                  kind="ExternalInput"),
            "dv1pk": nc.dram_tensor("dv1pk", [128, 3 * 128], BF16,
                                    kind="ExternalInput"),
            "dv1s": nc.dram_tensor("dv1s", [128, 3 * 128], BF16,
                                   kind="ExternalInput"),
            "wprojT": nc.dram_tensor("wprojT", [C, C], BF16,
                                     kind="ExternalInput"),
            "idf": nc.dram_tensor("idf", [128, 128], F32,
                                  kind="ExternalInput"),
            "tempb": nc.dram_tensor("tempb", [48, 4], F32,
                                    kind="ExternalInput"),
            "out": nc.dram_tensor("out", [C, OUT_ROWS, W], BF16,
                                  kind="ExternalOutput"),
        }
        with TileContext(nc) as tc:
            _program(nc, tc, io)
        nc.compile()
        _BUILT = nc
    return _BUILT


def _make_in_maps(x, w_qkv, w_dw, w_proj, temperature):
    wqkvT = np.zeros((256, 640), np.float32)
    wqkvT[0:C, 0:576] = w_qkv[:, :, 0, 0].T
    wqkvT = wqkvT.astype(BF16NP)
    wd = w_dw[:, 0, :, :].astype(np.float32)           # [576, 3, 3]
    dv0 = np.zeros((9, 128, 128), np.float32)
    for t, (dy, dx) in enumerate(TAPS):
        np.fill_diagonal(dv0[t], wd[384:512, dy, dx])
    dv0 = np.ascontiguousarray(
        dv0.transpose(1, 0, 2).reshape(128, 9 * 128)).astype(BF16NP)
    dv1pk = np.zeros((3, 128, 128), np.float32)
    for dx in range(3):
        np.fill_diagonal(dv1pk[dx, 0:64, 0:64], wd[512:576, 0, dx])
        np.fill_diagonal(dv1pk[dx, 64:128, 0:64], wd[512:576, 1, dx])
    dv1pk = np.ascontiguousarray(
        dv1pk.transpose(1, 0, 2).reshape(128, 3 * 128)).astype(BF16NP)
    dv1s = np.zeros((3, 128, 128), np.float32)
    for dx in range(3):
        np.fill_diagonal(dv1s[dx, 0:64, 0:64], wd[512:576, 2, dx])
    dv1s = np.ascontiguousarray(
        dv1s.transpose(1, 0, 2).reshape(128, 3 * 128)).astype(BF16NP)
    # fp8 DoubleRow diag weights for the q/k depthwise conv:
    # dqk8p pairs (dy0,dy1) per dx; dqk8d pairs (dy2,dx0)+(dy2,dx1) via
    # the in-SBUF replica; dqk8s is the lone (dy2,dx2) tap
    dqk8p = np.zeros((128, 9, 2, 128), np.float32)
    dqk8d = np.zeros((128, 3, 2, 128), np.float32)
    dqk8s = np.zeros((128, 3, 128), np.float32)
    for j in range(3):
        blk = wd[j * 128:(j + 1) * 128]                # [128, 3, 3]
        for dx in range(3):
            for i in range(2):
                np.fill_diagonal(dqk8p[:, j * 3 + dx, i, :], blk[:, i, dx])
        for i in range(2):
            np.fill_diagonal(dqk8d[:, j, i, :], blk[:, 2, i])
        np.fill_diagonal(dqk8s[:, j, :], blk[:, 2, 2])
    dqk8p = dqk8p.reshape(128, 9 * 256).astype(FP8NP)
    dqk8d = dqk8d.reshape(128, 3 * 256).astype(FP8NP)
    dqk8s = dqk8s.reshape(128, 3 * 128).astype(FP8NP)
    # fp8 paired-channel conv weights: wq8[k, i, m] = wqkvT[k + 96 i, m<384]
    wq8 = np.stack([w_qkv[:, :, 0, 0].T[:96, :384],
                    w_qkv[:, :, 0, 0].T[96:192, :384]], axis=1)
    wq8 = np.ascontiguousarray(wq8.astype(np.float32)).reshape(
        96, 2 * 384).astype(FP8NP)

    wprojT = np.ascontiguousarray(w_proj[:, :, 0, 0].T).astype(BF16NP)
    idf = np.eye(128, dtype=np.float32)
    tempb = np.ascontiguousarray(np.broadcast_to(
        np.asarray(temperature, np.float32).reshape(1, HEADS), (48, HEADS)))
    in_maps = []
    for core in range(8):
        b, s = core // 2, core % 2
        xs = np.zeros((C, SH_ROWS, W), BF16NP)
        r0 = s * OUT_ROWS - 1
        lo, hi = max(r0, 0), min(r0 + SH_ROWS, 128)
        xs[:, lo - r0: hi - r0, :] = x[b, :, lo:hi, :].astype(BF16NP)
        xf = xs.reshape(C, SH_ROWS * W).astype(np.float32)
        # channel pairs (c, c+96) interleaved per image row so the
        # DoubleRow pair step is 128 and inner runs stay 128 elements
        x8 = xf.reshape(2, 96, SH_ROWS, W).transpose(1, 2, 0, 3)
        x8 = np.ascontiguousarray(x8).reshape(96, 2 * NTOK).astype(FP8NP)
        in_maps.append({
            "x": xs, "wqkvT": wqkvT, "dqk8p": dqk8p, "dqk8d": dqk8d,
            "dqk8s": dqk8s,
            "x8": x8, "wq8": wq8, "dv0": dv0, "dv1pk": dv1pk, "dv1s": dv1s,
            "wprojT": wprojT, "idf": idf, "tempb": tempb,
        })
    return in_maps


def kernel(x, w_qkv, w_dw, w_proj, temperature):
    x = np.asarray(x, np.float32)
    nc = _get_built()
    in_maps = _make_in_maps(np.asarray(x, np.float32),
                            np.asarray(w_qkv, np.float32),
                            np.asarray(w_dw, np.float32),
                            np.asarray(w_proj, np.float32),
                            np.asarray(temperature, np.float32))
    res = run_bass_kernel_spmd(nc, in_maps, core_ids=list(range(8)))
    y = np.empty((4, C, 128, W), np.float32)
    for core in range(8):
        b, s = core // 2, core % 2
        y[b, :, s * OUT_ROWS:(s + 1) * OUT_ROWS, :] = np.asarray(
            res.results[core]["out"], np.float32)
    return y

